# revision 14
# baseline (speedup 1.0000x reference)
"""GQA attention kernel for Trainium2, 8 NeuronCores.

Problem: resid [2, 2048, 1024], 16 Q heads / 8 KV groups, d_head 64, causal,
out = softmax(QK^T/8 + causal) V -> W_out + b_out.

Sharding: tensor-parallel over (batch x kv-group-pairs). Core c handles
batch b = c // 4 and kv groups {2*(c%4), 2*(c%4)+1} = 4 Q heads. Each core
computes its heads' attention and a partial output projection; the host sums
the 4 partials per batch element and adds b_out.

Per-core dataflow (bf16 storage/matmuls, fp32 PSUM accumulation):
  - host passes resid[b].T in bf16; weights/resid arrive as one coalesced
    DMA each (many small DMAs serialize on the sync queue's ~0.6us issue)
  - K^T [128, S] projection and V [S, 2x65] (ones column appended per
    group -> the AV matmul produces sum-exp for free in output row 64) for
    ALL k tiles run upfront, overlapping the resid DMA stream and warming
    the PE clock gate
  - spans of 512 q rows are processed DESCENDING (3,2,1,0): the largest
    span's exp-heavy attention overlaps the remaining Q projections and
    deferred output-projection blocks, and the kernel tail is the smallest
    span's back-end instead of the largest's
  - scores computed transposed: S^T[k, q] = K @ Q^T; causality via q-start
    offset and an upper-triangular multiplicative mask on diagonal tiles
  - per k-tile the two heads of a group land in one 2-bank PSUM tile so a
    single ACT exp instruction (2D access pattern) covers both
  - softmax without max-subtraction (scores are O(1) by construction;
    masked lanes are exactly zero after the mask multiply)
  - U^T[e, q] accumulates per HALF-span into a 2-bank PSUM tile (4 head
    slots x 256 q), double-buffered, so the normalize chain of one half
    overlaps the attention of the next; sum-exp row 64 is one contiguous
    [1, 1024] AP -> one Ln + one Exp(-x) per half
  - a pre-placed ACT table load of natural_log_exp_and_others serves both
    Exp and Ln (the default pass thrashes 17 table loads otherwise)
  - normalize: partition-broadcast of 1/sumexp (GpSimd), multiply (DVE)
  - out_partial[s, d] = z^T.T @ W_out_stack accumulated over 2 e-chunks
  - a filler queue weaves next-processed-span Q projections and deferred
    normalize/output-projection work into the attention k-tile stream so
    the PE queue never blocks on the softmax denominators
"""

import sys

sys.path.insert(0, "/opt/trn_rl_repo")

from collections import deque

import ml_dtypes
import numpy as np

import concourse.bass as bass
import concourse.mybir as mybir
import concourse.tile as tile
from concourse import bacc
from concourse.bass_utils import run_bass_kernel_spmd
from concourse.hw_specs import get_activation_tables
from concourse.masks import make_upper_triangular

S = 2048          # seq len
D = 1024          # d_model
E = 64            # d_head
P = 128
NCHUNK = D // P   # 8 d_model chunks
SPAN = 512
HALF = 256
NSPAN = S // SPAN
NKT = S // P      # 16 k tiles
F32 = mybir.dt.float32
BF = mybir.dt.bfloat16
EXP = mybir.ActivationFunctionType.Exp
LN = mybir.ActivationFunctionType.Ln

LAST_RESULTS = None  # stashed BassKernelResults for the test harness
_CACHED_NC = None


def _build_program():
    nc = bacc.Bacc("TRN2", target_bir_lowering=False, debug=False)

    rT_d = nc.dram_tensor("resid_t", [D, S], BF, kind="ExternalInput")
    wq_d = nc.dram_tensor("wq", [D, 256], BF, kind="ExternalInput")
    wk_d = nc.dram_tensor("wk", [D, 128], BF, kind="ExternalInput")
    wv_d = nc.dram_tensor("wv", [D, 128], BF, kind="ExternalInput")
    wo_d = nc.dram_tensor("wo", [256, D], BF, kind="ExternalInput")
    out_d = nc.dram_tensor("out", [S, D], F32, kind="ExternalOutput")

    # one ACT table set serves Exp and Ln; pre-placing the load keeps the
    # fixpoint pass from alternating exp_and_others / natural_log sets
    tables = list(get_activation_tables(nc.m.arch).keys())
    nle_id = tables.index("natural_log_exp_and_others")

    with tile.TileContext(nc) as tc:
        with (
            tc.tile_pool(name="persist", bufs=1) as pp,
            tc.tile_pool(name="exp", bufs=6) as ep,
            tc.tile_pool(name="zt", bufs=2) as zp,
            tc.tile_pool(name="misc", bufs=2) as mp,
            tc.tile_pool(name="ostage", bufs=3) as op,
            tc.tile_pool(name="ps_u", bufs=2, space="PSUM") as ps_u,
            tc.tile_pool(name="ps_sc", bufs=2, space="PSUM") as ps_sc,
        ):
            nc.scalar.add_instruction(
                mybir.InstLoadActFuncSet(
                    name=nc.get_next_instruction_name(),
                    act_func_set_id=nle_id,
                )
            )

            # ---- coalesced weight loads (one DMA per weight tensor) ----
            wqb = pp.tile([P, NCHUNK * 256], BF, tag="wq")
            nc.sync.dma_start(
                wqb.rearrange("p (c n) -> p c n", c=NCHUNK),
                wq_d.rearrange("(c p) n -> p c n", p=P))
            wq_sb = [wqb[:, c * 256:(c + 1) * 256] for c in range(NCHUNK)]
            wkb = pp.tile([P, NCHUNK * 128], BF, tag="wk")
            nc.sync.dma_start(
                wkb.rearrange("p (c n) -> p c n", c=NCHUNK),
                wk_d.rearrange("(c p) n -> p c n", p=P))
            wk_sb = [wkb[:, c * 128:(c + 1) * 128] for c in range(NCHUNK)]
            wvb = pp.tile([P, NCHUNK * 128], BF, tag="wv")
            nc.sync.dma_start(
                wvb.rearrange("p (c n) -> p c n", c=NCHUNK),
                wv_d.rearrange("(c p) n -> p c n", p=P))
            wv_sb = [wvb[:, c * 128:(c + 1) * 128] for c in range(NCHUNK)]

            mask = pp.tile([P, P], BF, tag="mask")
            make_upper_triangular(nc, mask[:], val=1.0, diag=True)

            # residual: one 1MB DMA per span (all 8 d-chunks); wo last
            # (first needed by the output projection ~40us in)
            rTb = pp.tile([P, NCHUNK * S], BF, tag="rt")
            rT = [rTb[:, c * S:(c + 1) * S] for c in range(NCHUNK)]
            rTb_v = rTb.rearrange("p (c m) -> p c m", c=NCHUNK)
            rTd_v = rT_d.rearrange("(c p) m -> p c m", p=P)
            for sp in range(NSPAN):
                nc.sync.dma_start(
                    rTb_v[:, :, sp * SPAN:(sp + 1) * SPAN],
                    rTd_v[:, :, sp * SPAN:(sp + 1) * SPAN])
            wob = pp.tile([P, 2 * D], BF, tag="wo")
            nc.sync.dma_start(
                wob.rearrange("p (c n) -> p c n", c=2),
                wo_d.rearrange("(c p) n -> p c n", p=P))
            wo_sb = [wob[:, c * D:(c + 1) * D] for c in range(2)]

            qT = [pp.tile([P, S], BF, tag=f"qt{e}", name=f"qt{e}")
                  for e in range(2)]
            kT = pp.tile([P, S], BF, tag="kt")
            vaug = [pp.tile([P, 130], BF, tag=f"va{k}", name=f"va{k}")
                    for k in range(NKT)]
            for k in range(NKT):
                nc.gpsimd.memset(vaug[k][:, 64:65], 1.0)
                nc.gpsimd.memset(vaug[k][:, 129:130], 1.0)

            # ---- emitters ----
            def q_proj(sp, eblk):
                acc = ps_sc.tile([P, SPAN], F32, tag="sc", name="qacc")
                for c in range(NCHUNK):
                    nc.tensor.matmul(
                        acc[:],
                        wq_sb[c][:, eblk * P:(eblk + 1) * P],
                        rT[c][:, sp * SPAN:(sp + 1) * SPAN],
                        start=(c == 0),
                        stop=(c == NCHUNK - 1),
                    )
                nc.vector.tensor_copy(
                    qT[eblk][:, sp * SPAN:(sp + 1) * SPAN], acc[:])

            def k_proj(sp):
                acc = ps_sc.tile([P, SPAN], F32, tag="sc", name="kacc")
                for c in range(NCHUNK):
                    nc.tensor.matmul(
                        acc[:],
                        wk_sb[c][:],
                        rT[c][:, sp * SPAN:(sp + 1) * SPAN],
                        start=(c == 0),
                        stop=(c == NCHUNK - 1),
                    )
                nc.vector.tensor_copy(kT[:, sp * SPAN:(sp + 1) * SPAN], acc[:])

            def v_proj(kt):
                va = vaug[kt]
                acc = ps_sc.tile([P, 128], F32, tag="sc", name="vacc")
                for c in range(NCHUNK):
                    nc.tensor.matmul(
                        acc[:, 0:128],
                        rT[c][:, kt * P:(kt + 1) * P],
                        wv_sb[c][:],
                        start=(c == 0),
                        stop=(c == NCHUNK - 1),
                    )
                nc.vector.tensor_copy(va[:, 0:64], acc[:, 0:64])
                nc.vector.tensor_copy(va[:, 65:129], acc[:, 64:128])

            # u tile col layout: head slot s = 2g+i at cols s*HALF; row 64
            # of each slot is the sum-exp (ones column of vaug)
            def normalize(u_half, zc):
                # 1/x as exp(-ln x) on ScalarE, one op pair per half-span;
                # fp32 intermediates (bf16 ln costs ~2% in the exp back off)
                lnt = mp.tile([1, 4 * HALF], F32, tag="ln", name="lnt")
                nc.scalar.activation(lnt[:], u_half[64:65, :], LN)
                rec = mp.tile([1, 4 * HALF], F32, tag="rec", name="rec")
                nc.scalar.activation(rec[:], lnt[:], EXP, scale=-1.0)
                for g in range(2):
                    for i in range(2):
                        s_slot = 2 * g + i
                        bc = mp.tile([64, HALF], F32, tag=f"bc{s_slot}",
                                     name="bc")
                        nc.gpsimd.partition_broadcast(
                            bc[:],
                            rec[0:1, s_slot * HALF:(s_slot + 1) * HALF])
                        nc.vector.tensor_mul(
                            zc[i][g * 64:(g + 1) * 64, :],
                            u_half[0:64,
                                   s_slot * HALF:(s_slot + 1) * HALF],
                            bc[:],
                        )

            def o_proj_st(zc, s0):
                # one 128-row block of the output projection + store
                o_sb = op.tile([P, D], F32, tag="ost", name="osb")
                st = (s0 // P) % 2
                for dsp in range(2):
                    o_ps = ps_sc.tile([P, SPAN], F32, tag="sc", name="ops")
                    for ch in range(2):
                        nc.tensor.matmul(
                            o_ps[:],
                            zc[ch][:, st * P:(st + 1) * P],
                            wo_sb[ch][:, dsp * SPAN:(dsp + 1) * SPAN],
                            start=(ch == 0),
                            stop=(ch == 1),
                        )
                    nc.vector.tensor_copy(
                        o_sb[:, dsp * SPAN:(dsp + 1) * SPAN], o_ps[:])
                nc.sync.dma_start(out_d[s0:s0 + P, :], o_sb[:])

            # ---- upfront: K and V for every k tile (chained to the resid
            # span arrivals), then the first-processed span's Q ----
            for sp in range(NSPAN):
                k_proj(sp)
                for kt in range(4 * sp, 4 * sp + 4):
                    v_proj(kt)
            q_proj(NSPAN - 1, 0)
            q_proj(NSPAN - 1, 1)

            fillq = deque()
            q_left = [0] * NSPAN

            def q_thunks(sp):
                def wrap(e):
                    def run():
                        q_left[sp] -= 1
                        q_proj(sp, e)
                    return run
                q_left[sp] += 2
                return [wrap(0), wrap(1)]

            # ---- spans, largest first ----
            for sp in range(NSPAN - 1, -1, -1):
                # this span's Q must be emitted before its first scores
                while q_left[sp] > 0:
                    fillq.popleft()()
                if sp > 0:
                    fillq.extend(q_thunks(sp - 1))

                q0 = sp * SPAN
                nkt = (q0 + SPAN) // P   # k tiles touching this span
                hb = nkt - 2             # k tiles touching the low half
                u_halves = [
                    ps_u.tile([P, 4 * HALF], F32, tag="u", name=f"u{h}")
                    for h in range(2)
                ]
                zcs = [
                    [zp.tile([P, HALF], BF, tag=f"zt{h}{c}", name=f"z{c}")
                     for c in range(2)]
                    for h in range(2)
                ]

                # AV of k-tile kt is emitted after the scores+exp of kt+1,
                # hiding the ACT exp latency from the PE stream; each AV
                # batch entry splits over the two half-span u tiles
                def emit_av(batch):
                    for g, i, e_sb, kt_, off_, w_ in batch:
                        s_slot = 2 * g + i
                        for h in range(2):
                            lo = max(off_ - h * HALF, 0)
                            hi = min(off_ + w_ - h * HALF, HALF)
                            if lo >= hi:
                                continue
                            last_kt = hb - 1 if h == 0 else nkt - 1
                            # two head slots share each PSUM bank and
                            # start=True clears has_written for the WHOLE
                            # bank: only the bank's first slot may set it
                            # (the sibling's first write lands on cleared
                            # bits, which means overwrite -> still correct)
                            nc.tensor.matmul(
                                u_halves[h][0:65,
                                            s_slot * HALF + lo:
                                            s_slot * HALF + hi],
                                vaug[kt_][:, g * 65:(g + 1) * 65],
                                e_sb[:, i * SPAN + h * HALF + lo:
                                     i * SPAN + h * HALF + hi],
                                start=(kt_ == 0 and i == 0),
                                stop=(kt_ == last_kt),
                                skip_group_check=True,
                            )

                pending = []
                for kt in range(nkt):
                    k0 = kt * P
                    off = max(k0 - q0, 0)
                    w = SPAN - off
                    cur = []
                    for g in range(2):
                        # both i-heads of group g share one 2-bank PSUM
                        # tile -> a single exp instruction covers them
                        pair = ps_sc.tile([P, 2 * SPAN], F32, tag="sc",
                                          name=f"pair{g}")
                        for i in range(2):
                            nc.tensor.matmul(
                                pair[:, i * SPAN + off:i * SPAN + off + w],
                                kT[g * 64:(g + 1) * 64, k0:k0 + P],
                                qT[i][g * 64:(g + 1) * 64,
                                      q0 + off:q0 + off + w],
                                start=True,
                                stop=True,
                            )
                        e_sb = ep.tile([P, 2 * SPAN], BF, tag="e",
                                       name=f"e{g}")
                        pv = pair.rearrange("p (i w) -> p i w", i=2)
                        ev = e_sb.rearrange("p (i w) -> p i w", i=2)
                        nc.scalar.activation(
                            ev[:, :, off:off + w], pv[:, :, off:off + w],
                            EXP, scale=0.125,
                        )
                        if k0 >= q0:  # diagonal tile -> causal mask
                            mv = mask.unsqueeze(1).broadcast_to([P, 2, P])
                            nc.vector.tensor_mul(
                                ev[:, :, off:off + P],
                                ev[:, :, off:off + P],
                                mv,
                            )
                        cur.append((g, 0, e_sb, kt, off, w))
                        cur.append((g, 1, e_sb, kt, off, w))
                    emit_av(pending)
                    pending = cur
                    if kt == hb:
                        # low half's AV chain (k tiles 0..hb-1) just went
                        # out -> emit its normalize now, queue its O-proj
                        normalize(u_halves[0], zcs[0])
                        fillq.append(
                            lambda z=zcs[0], s=q0: o_proj_st(z, s))
                        fillq.append(
                            lambda z=zcs[0], s=q0 + P: o_proj_st(z, s))
                    if fillq:
                        fillq.popleft()()
                emit_av(pending)

                hq1 = q0 + HALF
                if sp > 0:
                    fillq.append(
                        lambda u=u_halves[1], z=zcs[1]: normalize(u, z))
                    fillq.append(lambda z=zcs[1], s=hq1: o_proj_st(z, s))
                    fillq.append(
                        lambda z=zcs[1], s=hq1 + P: o_proj_st(z, s))
                else:
                    normalize(u_halves[1], zcs[1])
                    while fillq:
                        fillq.popleft()()
                    o_proj_st(zcs[1], hq1)
                    o_proj_st(zcs[1], hq1 + P)

    nc.finalize()
    return nc


def kernel(resid, W_Q, W_K, W_V, W_out, b_out):
    global LAST_RESULTS, _CACHED_NC
    resid = np.asarray(resid, np.float32)
    W_Q = np.asarray(W_Q, np.float32)
    W_K = np.asarray(W_K, np.float32)
    W_V = np.asarray(W_V, np.float32)
    W_out = np.asarray(W_out, np.float32)
    b_out = np.asarray(b_out, np.float32)

    if _CACHED_NC is None:
        _CACHED_NC = _build_program()
    nc = _CACHED_NC

    bf = ml_dtypes.bfloat16
    residT = [np.ascontiguousarray(resid[b].T).astype(bf) for b in range(2)]
    in_maps = []
    for c in range(8):
        b, q = c // 4, c % 4
        # interleaved head order [h0, h2, h1, h3]: storage slot (g, i) holds
        # local head 2g+i -> qT[i]/zc[i] rows g*64 (see _build_program)
        heads = [4 * q, 4 * q + 2, 4 * q + 1, 4 * q + 3]
        groups = [2 * q, 2 * q + 1]
        in_maps.append({
            "resid_t": residT[b],
            "wq": np.ascontiguousarray(
                W_Q[:, heads, :].reshape(D, 256)).astype(bf),
            "wk": np.ascontiguousarray(
                W_K[:, groups, :].reshape(D, 128)).astype(bf),
            "wv": np.ascontiguousarray(
                W_V[:, groups, :].reshape(D, 128)).astype(bf),
            "wo": np.ascontiguousarray(
                W_out[:, heads, :].transpose(1, 0, 2).reshape(256, D)
            ).astype(bf),
        })

    res = run_bass_kernel_spmd(nc, in_maps, core_ids=list(range(8)))
    LAST_RESULTS = res

    out = np.zeros((2, S, D), np.float32)
    for c in range(8):
        out[c // 4] += res.results[c]["out"]
    out += b_out
    return out


# revision 16
# speedup vs baseline: 1.1261x; 1.1261x over previous
"""GQA attention kernel for Trainium2, 8 NeuronCores.

Problem: resid [2, 2048, 1024], 16 Q heads / 8 KV groups, d_head 64, causal,
out = softmax(QK^T/8 + causal) V -> W_out + b_out.

Sharding: tensor-parallel over (batch x kv-group-pairs). Core c handles
batch b = c // 4 and kv groups {2*(c%4), 2*(c%4)+1} = 4 Q heads. Each core
computes its heads' attention and a partial output projection; the host sums
the 4 partials per batch element and adds b_out.

Per-core dataflow (bf16 storage/matmuls, fp32 PSUM accumulation):
  - host passes resid[b].T in bf16; weights/resid arrive as one coalesced
    DMA each (many small DMAs serialize on the sync queue's ~0.6us issue)
  - K^T [128, S] projection and V [S, 2x65] (ones column appended per
    group -> the AV matmul produces sum-exp for free in output row 64) for
    ALL k tiles run upfront, overlapping the resid DMA stream and warming
    the PE clock gate
  - spans of 512 q rows are processed DESCENDING (3,2,1,0): the largest
    span's exp-heavy attention overlaps the remaining Q projections and
    deferred output-projection blocks, and the kernel tail is the smallest
    span's back-end instead of the largest's
  - scores computed transposed: S^T[k, q] = K @ Q^T; causality via q-start
    offset and an upper-triangular multiplicative mask on diagonal tiles
  - per k-tile the two heads of a group land in one 2-bank PSUM tile so a
    single ACT exp instruction (2D access pattern) covers both
  - softmax without max-subtraction (scores are O(1) by construction;
    masked lanes are exactly zero after the mask multiply)
  - U^T[e, q] accumulates per HALF-span into a 2-bank PSUM tile (4 head
    slots x 256 q), double-buffered, so the normalize chain of one half
    overlaps the attention of the next; sum-exp row 64 is one contiguous
    [1, 1024] AP -> one Ln + one Exp(-x) per half
  - a pre-placed ACT table load of natural_log_exp_and_others serves both
    Exp and Ln (the default pass thrashes 17 table loads otherwise)
  - normalize: partition-broadcast of 1/sumexp (GpSimd), multiply (DVE)
  - out_partial[s, d] = z^T.T @ W_out_stack accumulated over 2 e-chunks
  - a filler queue weaves next-processed-span Q projections and deferred
    normalize/output-projection work into the attention k-tile stream so
    the PE queue never blocks on the softmax denominators
"""

import sys

sys.path.insert(0, "/opt/trn_rl_repo")

from collections import deque

import ml_dtypes
import numpy as np

import concourse.bass as bass
import concourse.mybir as mybir
import concourse.tile as tile
from concourse import bacc
from concourse.bass_utils import run_bass_kernel_spmd
from concourse.hw_specs import get_activation_tables
from concourse.masks import make_upper_triangular

S = 2048          # seq len
D = 1024          # d_model
E = 64            # d_head
P = 128
NCHUNK = D // P   # 8 d_model chunks
SPAN = 512
HALF = 256
NSPAN = S // SPAN
NKT = S // P      # 16 k tiles
F32 = mybir.dt.float32
BF = mybir.dt.bfloat16
EXP = mybir.ActivationFunctionType.Exp
LN = mybir.ActivationFunctionType.Ln

LAST_RESULTS = None  # stashed BassKernelResults for the test harness
_CACHED_NC = None


def _build_program():
    nc = bacc.Bacc("TRN2", target_bir_lowering=False, debug=False)

    rT_d = nc.dram_tensor("resid_t", [D, S], BF, kind="ExternalInput")
    wq_d = nc.dram_tensor("wq", [D, 256], BF, kind="ExternalInput")
    wk_d = nc.dram_tensor("wk", [D, 128], BF, kind="ExternalInput")
    wv_d = nc.dram_tensor("wv", [D, 128], BF, kind="ExternalInput")
    wo_d = nc.dram_tensor("wo", [256, D], BF, kind="ExternalInput")
    out_d = nc.dram_tensor("out", [S, D], F32, kind="ExternalOutput")

    # one ACT table set serves Exp and Ln; pre-placing the load keeps the
    # fixpoint pass from alternating exp_and_others / natural_log sets
    tables = list(get_activation_tables(nc.m.arch).keys())
    nle_id = tables.index("natural_log_exp_and_others")

    with tile.TileContext(nc) as tc:
        with (
            tc.tile_pool(name="persist", bufs=1) as pp,
            tc.tile_pool(name="exp", bufs=6) as ep,
            tc.tile_pool(name="zt", bufs=2) as zp,
            tc.tile_pool(name="misc", bufs=2) as mp,
            tc.tile_pool(name="ostage", bufs=3) as op,
            tc.tile_pool(name="ps_u", bufs=2, space="PSUM") as ps_u,
            tc.tile_pool(name="ps_sc", bufs=2, space="PSUM") as ps_sc,
        ):
            nc.scalar.add_instruction(
                mybir.InstLoadActFuncSet(
                    name=nc.get_next_instruction_name(),
                    act_func_set_id=nle_id,
                )
            )

            # ---- coalesced weight loads (one DMA per weight tensor) ----
            wqb = pp.tile([P, NCHUNK * 256], BF, tag="wq")
            nc.sync.dma_start(
                wqb.rearrange("p (c n) -> p c n", c=NCHUNK),
                wq_d.rearrange("(c p) n -> p c n", p=P))
            wq_sb = [wqb[:, c * 256:(c + 1) * 256] for c in range(NCHUNK)]
            wkb = pp.tile([P, NCHUNK * 128], BF, tag="wk")
            nc.sync.dma_start(
                wkb.rearrange("p (c n) -> p c n", c=NCHUNK),
                wk_d.rearrange("(c p) n -> p c n", p=P))
            wk_sb = [wkb[:, c * 128:(c + 1) * 128] for c in range(NCHUNK)]
            wvb = pp.tile([P, NCHUNK * 128], BF, tag="wv")
            nc.sync.dma_start(
                wvb.rearrange("p (c n) -> p c n", c=NCHUNK),
                wv_d.rearrange("(c p) n -> p c n", p=P))
            wv_sb = [wvb[:, c * 128:(c + 1) * 128] for c in range(NCHUNK)]

            mask = pp.tile([P, P], BF, tag="mask")
            make_upper_triangular(nc, mask[:], val=1.0, diag=True)

            # residual: one 1MB DMA per span (all 8 d-chunks); wo last
            # (first needed by the output projection ~40us in)
            rTb = pp.tile([P, NCHUNK * S], BF, tag="rt")
            rT = [rTb[:, c * S:(c + 1) * S] for c in range(NCHUNK)]
            rTb_v = rTb.rearrange("p (c m) -> p c m", c=NCHUNK)
            rTd_v = rT_d.rearrange("(c p) m -> p c m", p=P)
            for sp in range(NSPAN):
                nc.sync.dma_start(
                    rTb_v[:, :, sp * SPAN:(sp + 1) * SPAN],
                    rTd_v[:, :, sp * SPAN:(sp + 1) * SPAN])
            wob = pp.tile([P, 2 * D], BF, tag="wo")
            nc.sync.dma_start(
                wob.rearrange("p (c n) -> p c n", c=2),
                wo_d.rearrange("(c p) n -> p c n", p=P))
            wo_sb = [wob[:, c * D:(c + 1) * D] for c in range(2)]

            qT = [pp.tile([P, S], BF, tag=f"qt{e}", name=f"qt{e}")
                  for e in range(2)]
            kT = pp.tile([P, S], BF, tag="kt")
            vaug = [pp.tile([P, 130], BF, tag=f"va{k}", name=f"va{k}")
                    for k in range(NKT)]
            for k in range(NKT):
                nc.gpsimd.memset(vaug[k][:, 64:65], 1.0)
                nc.gpsimd.memset(vaug[k][:, 129:130], 1.0)

            # ---- emitters ----
            def q_proj(sp, eblk):
                acc = ps_sc.tile([P, SPAN], F32, tag="sc", name="qacc")
                for c in range(NCHUNK):
                    nc.tensor.matmul(
                        acc[:],
                        wq_sb[c][:, eblk * P:(eblk + 1) * P],
                        rT[c][:, sp * SPAN:(sp + 1) * SPAN],
                        start=(c == 0),
                        stop=(c == NCHUNK - 1),
                    )
                nc.vector.tensor_copy(
                    qT[eblk][:, sp * SPAN:(sp + 1) * SPAN], acc[:])

            def k_proj(sp):
                acc = ps_sc.tile([P, SPAN], F32, tag="sc", name="kacc")
                for c in range(NCHUNK):
                    nc.tensor.matmul(
                        acc[:],
                        wk_sb[c][:],
                        rT[c][:, sp * SPAN:(sp + 1) * SPAN],
                        start=(c == 0),
                        stop=(c == NCHUNK - 1),
                    )
                nc.vector.tensor_copy(kT[:, sp * SPAN:(sp + 1) * SPAN], acc[:])

            def v_proj(kt):
                va = vaug[kt]
                acc = ps_sc.tile([P, 128], F32, tag="sc", name="vacc")
                for c in range(NCHUNK):
                    nc.tensor.matmul(
                        acc[:, 0:128],
                        rT[c][:, kt * P:(kt + 1) * P],
                        wv_sb[c][:],
                        start=(c == 0),
                        stop=(c == NCHUNK - 1),
                    )
                nc.vector.tensor_copy(va[:, 0:64], acc[:, 0:64])
                nc.vector.tensor_copy(va[:, 65:129], acc[:, 64:128])

            # u tile col layout: head slot s = 2g+i at cols s*HALF; row 64
            # of each slot is the sum-exp (ones column of vaug)
            def normalize(u_half, zc):
                # 1/x as exp(-ln x) on ScalarE, one op pair per half-span;
                # fp32 intermediates (bf16 ln costs ~2% in the exp back off)
                lnt = mp.tile([1, 4 * HALF], F32, tag="ln", name="lnt")
                nc.scalar.activation(lnt[:], u_half[64:65, :], LN)
                rec = mp.tile([1, 4 * HALF], F32, tag="rec", name="rec")
                nc.scalar.activation(rec[:], lnt[:], EXP, scale=-1.0)
                for g in range(2):
                    for i in range(2):
                        s_slot = 2 * g + i
                        bc = mp.tile([64, HALF], F32, tag=f"bc{s_slot}",
                                     name="bc")
                        nc.gpsimd.partition_broadcast(
                            bc[:],
                            rec[0:1, s_slot * HALF:(s_slot + 1) * HALF])
                        nc.vector.tensor_mul(
                            zc[i][g * 64:(g + 1) * 64, :],
                            u_half[0:64,
                                   s_slot * HALF:(s_slot + 1) * HALF],
                            bc[:],
                        )

            def o_proj_st(zc, s0):
                # one 128-row block of the output projection + store
                o_sb = op.tile([P, D], F32, tag="ost", name="osb")
                st = (s0 // P) % 2
                for dsp in range(2):
                    o_ps = ps_sc.tile([P, SPAN], F32, tag="sc", name="ops")
                    for ch in range(2):
                        nc.tensor.matmul(
                            o_ps[:],
                            zc[ch][:, st * P:(st + 1) * P],
                            wo_sb[ch][:, dsp * SPAN:(dsp + 1) * SPAN],
                            start=(ch == 0),
                            stop=(ch == 1),
                        )
                    nc.vector.tensor_copy(
                        o_sb[:, dsp * SPAN:(dsp + 1) * SPAN], o_ps[:])
                nc.sync.dma_start(out_d[s0:s0 + P, :], o_sb[:])

            proj_left = [0] * (NSPAN + 1)

            def proj_thunks(sp):
                def wrap(fn):
                    def run():
                        proj_left[sp] -= 1
                        fn()
                    return run
                th = [wrap(lambda e=e: q_proj(sp, e)) for e in range(2)]
                th.append(wrap(lambda: k_proj(sp)))
                th += [wrap(lambda kt=kt: v_proj(kt))
                       for kt in range(4 * sp, 4 * sp + 4)]
                proj_left[sp] += len(th)
                return th

            fillq = deque()

            # span 0 projections run upfront (overlap the resid DMA tail)
            for th in proj_thunks(0):
                th()

            for sp in range(NSPAN):
                # this span's own projections MUST be emitted before its
                # first score matmuls read qT/kT/vaug (emission order is
                # dataflow order for a fixed SBUF slice)
                while proj_left[sp] > 0:
                    fillq.popleft()()
                if sp + 1 < NSPAN:
                    fillq.extend(proj_thunks(sp + 1))

                q0 = sp * SPAN
                nkt = (q0 + SPAN) // P   # k tiles touching this span
                hb = nkt - 2             # k tiles touching the low half
                u_halves = [
                    ps_u.tile([P, 4 * HALF], F32, tag="u", name=f"u{h}")
                    for h in range(2)
                ]
                zcs = [
                    [zp.tile([P, HALF], BF, tag=f"zt{h}{c}", name=f"z{c}")
                     for c in range(2)]
                    for h in range(2)
                ]

                # AV of k-tile kt is emitted after the scores+exp of kt+1,
                # hiding the ACT exp latency from the PE stream; each AV
                # batch entry splits over the two half-span u tiles
                def emit_av(batch):
                    for g, i, e_sb, kt_, off_, w_ in batch:
                        s_slot = 2 * g + i
                        for h in range(2):
                            lo = max(off_ - h * HALF, 0)
                            hi = min(off_ + w_ - h * HALF, HALF)
                            if lo >= hi:
                                continue
                            last_kt = hb - 1 if h == 0 else nkt - 1
                            # two head slots share each PSUM bank and
                            # start=True clears has_written for the WHOLE
                            # bank: only the bank's first slot may set it
                            # (the sibling's first write lands on cleared
                            # bits, which means overwrite -> still correct)
                            nc.tensor.matmul(
                                u_halves[h][0:65,
                                            s_slot * HALF + lo:
                                            s_slot * HALF + hi],
                                vaug[kt_][:, g * 65:(g + 1) * 65],
                                e_sb[:, i * SPAN + h * HALF + lo:
                                     i * SPAN + h * HALF + hi],
                                start=(kt_ == 0 and i == 0),
                                stop=(kt_ == last_kt),
                                skip_group_check=True,
                            )

                pending = []
                for kt in range(nkt):
                    k0 = kt * P
                    off = max(k0 - q0, 0)
                    w = SPAN - off
                    cur = []
                    for g in range(2):
                        # both i-heads of group g share one 2-bank PSUM
                        # tile -> a single exp instruction covers them
                        pair = ps_sc.tile([P, 2 * SPAN], F32, tag="sc",
                                          name=f"pair{g}")
                        for i in range(2):
                            nc.tensor.matmul(
                                pair[:, i * SPAN + off:i * SPAN + off + w],
                                kT[g * 64:(g + 1) * 64, k0:k0 + P],
                                qT[i][g * 64:(g + 1) * 64,
                                      q0 + off:q0 + off + w],
                                start=True,
                                stop=True,
                            )
                        e_sb = ep.tile([P, 2 * SPAN], BF, tag="e",
                                       name=f"e{g}")
                        pv = pair.rearrange("p (i w) -> p i w", i=2)
                        ev = e_sb.rearrange("p (i w) -> p i w", i=2)
                        nc.scalar.activation(
                            ev[:, :, off:off + w], pv[:, :, off:off + w],
                            EXP, scale=0.125,
                        )
                        if k0 >= q0:  # diagonal tile -> causal mask
                            mv = mask.unsqueeze(1).broadcast_to([P, 2, P])
                            nc.vector.tensor_mul(
                                ev[:, :, off:off + P],
                                ev[:, :, off:off + P],
                                mv,
                            )
                        cur.append((g, 0, e_sb, kt, off, w))
                        cur.append((g, 1, e_sb, kt, off, w))
                    emit_av(pending)
                    pending = cur
                    if kt == hb:
                        # low half's AV chain (k tiles 0..hb-1) just went
                        # out -> emit its normalize now, queue its O-proj
                        normalize(u_halves[0], zcs[0])
                        fillq.append(
                            lambda z=zcs[0], s=q0: o_proj_st(z, s))
                        fillq.append(
                            lambda z=zcs[0], s=q0 + P: o_proj_st(z, s))
                    if fillq:
                        fillq.popleft()()
                emit_av(pending)

                hq1 = q0 + HALF
                if sp + 1 < NSPAN:
                    fillq.append(
                        lambda u=u_halves[1], z=zcs[1]: normalize(u, z))
                    fillq.append(lambda z=zcs[1], s=hq1: o_proj_st(z, s))
                    fillq.append(
                        lambda z=zcs[1], s=hq1 + P: o_proj_st(z, s))
                else:
                    normalize(u_halves[1], zcs[1])
                    while fillq:
                        fillq.popleft()()
                    o_proj_st(zcs[1], hq1)
                    o_proj_st(zcs[1], hq1 + P)

    nc.finalize()
    return nc


def kernel(resid, W_Q, W_K, W_V, W_out, b_out):
    global LAST_RESULTS, _CACHED_NC
    resid = np.asarray(resid, np.float32)
    W_Q = np.asarray(W_Q, np.float32)
    W_K = np.asarray(W_K, np.float32)
    W_V = np.asarray(W_V, np.float32)
    W_out = np.asarray(W_out, np.float32)
    b_out = np.asarray(b_out, np.float32)

    if _CACHED_NC is None:
        _CACHED_NC = _build_program()
    nc = _CACHED_NC

    bf = ml_dtypes.bfloat16
    residT = [np.ascontiguousarray(resid[b].T).astype(bf) for b in range(2)]
    in_maps = []
    for c in range(8):
        b, q = c // 4, c % 4
        # interleaved head order [h0, h2, h1, h3]: storage slot (g, i) holds
        # local head 2g+i -> qT[i]/zc[i] rows g*64 (see _build_program)
        heads = [4 * q, 4 * q + 2, 4 * q + 1, 4 * q + 3]
        groups = [2 * q, 2 * q + 1]
        in_maps.append({
            "resid_t": residT[b],
            "wq": np.ascontiguousarray(
                W_Q[:, heads, :].reshape(D, 256)).astype(bf),
            "wk": np.ascontiguousarray(
                W_K[:, groups, :].reshape(D, 128)).astype(bf),
            "wv": np.ascontiguousarray(
                W_V[:, groups, :].reshape(D, 128)).astype(bf),
            "wo": np.ascontiguousarray(
                W_out[:, heads, :].transpose(1, 0, 2).reshape(256, D)
            ).astype(bf),
        })

    res = run_bass_kernel_spmd(nc, in_maps, core_ids=list(range(8)))
    LAST_RESULTS = res

    out = np.zeros((2, S, D), np.float32)
    for c in range(8):
        out[c // 4] += res.results[c]["out"]
    out += b_out
    return out


# revision 18
# speedup vs baseline: 1.1388x; 1.0113x over previous
"""GQA attention kernel for Trainium2, 8 NeuronCores.

Problem: resid [2, 2048, 1024], 16 Q heads / 8 KV groups, d_head 64, causal,
out = softmax(QK^T/8 + causal) V -> W_out + b_out.

Sharding: tensor-parallel over (batch x kv-group-pairs). Core c handles
batch b = c // 4 and kv groups {2*(c%4), 2*(c%4)+1} = 4 Q heads. Each core
computes its heads' attention and a partial output projection; the host sums
the 4 partials per batch element and adds b_out.

Per-core dataflow (bf16 storage/matmuls, fp32 PSUM accumulation):
  - host passes resid[b].T in bf16; weights/resid arrive as one coalesced
    DMA each (many small DMAs serialize on the sync queue's ~0.6us issue)
  - K^T [128, S] projection and V [S, 2x65] (ones column appended per
    group -> the AV matmul produces sum-exp for free in output row 64) for
    ALL k tiles run upfront, overlapping the resid DMA stream and warming
    the PE clock gate
  - spans of 512 q rows are processed DESCENDING (3,2,1,0): the largest
    span's exp-heavy attention overlaps the remaining Q projections and
    deferred output-projection blocks, and the kernel tail is the smallest
    span's back-end instead of the largest's
  - scores computed transposed: S^T[k, q] = K @ Q^T; causality via q-start
    offset and an upper-triangular multiplicative mask on diagonal tiles
  - per k-tile the two heads of a group land in one 2-bank PSUM tile so a
    single ACT exp instruction (2D access pattern) covers both
  - softmax without max-subtraction (scores are O(1) by construction;
    masked lanes are exactly zero after the mask multiply)
  - U^T[e, q] accumulates per HALF-span into a 2-bank PSUM tile (4 head
    slots x 256 q), double-buffered, so the normalize chain of one half
    overlaps the attention of the next; sum-exp row 64 is one contiguous
    [1, 1024] AP -> one Ln + one Exp(-x) per half
  - a pre-placed ACT table load of natural_log_exp_and_others serves both
    Exp and Ln (the default pass thrashes 17 table loads otherwise)
  - normalize: partition-broadcast of 1/sumexp (GpSimd), multiply (DVE)
  - out_partial[s, d] = z^T.T @ W_out_stack accumulated over 2 e-chunks
  - a filler queue weaves next-processed-span Q projections and deferred
    normalize/output-projection work into the attention k-tile stream so
    the PE queue never blocks on the softmax denominators
"""

import sys

sys.path.insert(0, "/opt/trn_rl_repo")

from collections import deque

import ml_dtypes
import numpy as np

import concourse.bass as bass
import concourse.mybir as mybir
import concourse.tile as tile
from concourse import bacc
from concourse.bass_utils import run_bass_kernel_spmd
from concourse.hw_specs import get_activation_tables
from concourse.masks import make_upper_triangular

S = 2048          # seq len
D = 1024          # d_model
E = 64            # d_head
P = 128
NCHUNK = D // P   # 8 d_model chunks
SPAN = 512
HALF = 256
NSPAN = S // SPAN
NKT = S // P      # 16 k tiles
F32 = mybir.dt.float32
BF = mybir.dt.bfloat16
EXP = mybir.ActivationFunctionType.Exp
LN = mybir.ActivationFunctionType.Ln

LAST_RESULTS = None  # stashed BassKernelResults for the test harness
_CACHED_NC = None


def _build_program():
    nc = bacc.Bacc("TRN2", target_bir_lowering=False, debug=False)

    rT_d = nc.dram_tensor("resid_t", [D, S], BF, kind="ExternalInput")
    wq_d = nc.dram_tensor("wq", [D, 256], BF, kind="ExternalInput")
    wk_d = nc.dram_tensor("wk", [D, 128], BF, kind="ExternalInput")
    wv_d = nc.dram_tensor("wv", [D, 128], BF, kind="ExternalInput")
    wo_d = nc.dram_tensor("wo", [256, D], BF, kind="ExternalInput")
    out_d = nc.dram_tensor("out", [S, D], F32, kind="ExternalOutput")

    # one ACT table set serves Exp and Ln; pre-placing the load keeps the
    # fixpoint pass from alternating exp_and_others / natural_log sets
    tables = list(get_activation_tables(nc.m.arch).keys())
    nle_id = tables.index("natural_log_exp_and_others")

    with tile.TileContext(nc) as tc:
        with (
            tc.tile_pool(name="persist", bufs=1) as pp,
            tc.tile_pool(name="exp", bufs=6) as ep,
            tc.tile_pool(name="zt", bufs=2) as zp,
            tc.tile_pool(name="misc", bufs=2) as mp,
            tc.tile_pool(name="ostage", bufs=3) as op,
            tc.tile_pool(name="ps_u", bufs=2, space="PSUM") as ps_u,
            tc.tile_pool(name="ps_sc", bufs=2, space="PSUM") as ps_sc,
        ):
            nc.scalar.add_instruction(
                mybir.InstLoadActFuncSet(
                    name=nc.get_next_instruction_name(),
                    act_func_set_id=nle_id,
                )
            )

            # residual first on the sync HWDGE ring: one 1MB DMA per span
            # (each DMA issue costs ~0.9us serially on its engine, so the
            # resid stream must head the queue)
            rTb = pp.tile([P, NCHUNK * S], BF, tag="rt")
            rT = [rTb[:, c * S:(c + 1) * S] for c in range(NCHUNK)]
            rTb_v = rTb.rearrange("p (c m) -> p c m", c=NCHUNK)
            rTd_v = rT_d.rearrange("(c p) m -> p c m", p=P)
            for sp in range(NSPAN):
                nc.sync.dma_start(
                    rTb_v[:, :, sp * SPAN:(sp + 1) * SPAN],
                    rTd_v[:, :, sp * SPAN:(sp + 1) * SPAN])

            # weights ride the scalar engine's HWDGE ring concurrently
            wqb = pp.tile([P, NCHUNK * 256], BF, tag="wq")
            nc.scalar.dma_start(
                wqb.rearrange("p (c n) -> p c n", c=NCHUNK),
                wq_d.rearrange("(c p) n -> p c n", p=P))
            wq_sb = [wqb[:, c * 256:(c + 1) * 256] for c in range(NCHUNK)]
            wkb = pp.tile([P, NCHUNK * 128], BF, tag="wk")
            nc.scalar.dma_start(
                wkb.rearrange("p (c n) -> p c n", c=NCHUNK),
                wk_d.rearrange("(c p) n -> p c n", p=P))
            wk_sb = [wkb[:, c * 128:(c + 1) * 128] for c in range(NCHUNK)]
            wvb = pp.tile([P, NCHUNK * 128], BF, tag="wv")
            nc.scalar.dma_start(
                wvb.rearrange("p (c n) -> p c n", c=NCHUNK),
                wv_d.rearrange("(c p) n -> p c n", p=P))
            wv_sb = [wvb[:, c * 128:(c + 1) * 128] for c in range(NCHUNK)]
            wob = pp.tile([P, 2 * D], BF, tag="wo")
            nc.scalar.dma_start(
                wob.rearrange("p (c n) -> p c n", c=2),
                wo_d.rearrange("(c p) n -> p c n", p=P))
            wo_sb = [wob[:, c * D:(c + 1) * D] for c in range(2)]

            mask = pp.tile([P, P], BF, tag="mask")
            make_upper_triangular(nc, mask[:], val=1.0, diag=True)

            # warm-up matmuls on throwaway data: the PE clock gate needs
            # ~3.4us of sustained activity to reach 2.4 GHz; burn the
            # initial DMA wait instead of ramping on real work
            warm = ps_sc.tile([P, P], F32, tag="sc", name="warm")
            for r in range(32):
                nc.tensor.matmul(warm[:], mask[:], mask[:],
                                 start=(r == 0), stop=(r == 31))

            qT = [pp.tile([P, S], BF, tag=f"qt{e}", name=f"qt{e}")
                  for e in range(2)]
            kT = pp.tile([P, S], BF, tag="kt")
            vaug = [pp.tile([P, 130], BF, tag=f"va{k}", name=f"va{k}")
                    for k in range(NKT)]
            for k in range(NKT):
                nc.gpsimd.memset(vaug[k][:, 64:65], 1.0)
                nc.gpsimd.memset(vaug[k][:, 129:130], 1.0)

            # ---- emitters ----
            def q_proj(sp, eblk):
                acc = ps_sc.tile([P, SPAN], F32, tag="sc", name="qacc")
                for c in range(NCHUNK):
                    nc.tensor.matmul(
                        acc[:],
                        wq_sb[c][:, eblk * P:(eblk + 1) * P],
                        rT[c][:, sp * SPAN:(sp + 1) * SPAN],
                        start=(c == 0),
                        stop=(c == NCHUNK - 1),
                    )
                nc.vector.tensor_copy(
                    qT[eblk][:, sp * SPAN:(sp + 1) * SPAN], acc[:])

            def k_proj(sp):
                acc = ps_sc.tile([P, SPAN], F32, tag="sc", name="kacc")
                for c in range(NCHUNK):
                    nc.tensor.matmul(
                        acc[:],
                        wk_sb[c][:],
                        rT[c][:, sp * SPAN:(sp + 1) * SPAN],
                        start=(c == 0),
                        stop=(c == NCHUNK - 1),
                    )
                nc.vector.tensor_copy(kT[:, sp * SPAN:(sp + 1) * SPAN], acc[:])

            def v_proj(kt):
                va = vaug[kt]
                acc = ps_sc.tile([P, 128], F32, tag="sc", name="vacc")
                for c in range(NCHUNK):
                    nc.tensor.matmul(
                        acc[:, 0:128],
                        rT[c][:, kt * P:(kt + 1) * P],
                        wv_sb[c][:],
                        start=(c == 0),
                        stop=(c == NCHUNK - 1),
                    )
                nc.vector.tensor_copy(va[:, 0:64], acc[:, 0:64])
                nc.vector.tensor_copy(va[:, 65:129], acc[:, 64:128])

            # u tile col layout: head slot s = 2g+i at cols s*HALF; row 64
            # of each slot is the sum-exp (ones column of vaug)
            def normalize(u_half, zc):
                # 1/x as exp(-ln x) on ScalarE, one op pair per half-span;
                # fp32 intermediates (bf16 ln costs ~2% in the exp back off)
                lnt = mp.tile([1, 4 * HALF], F32, tag="ln", name="lnt")
                nc.scalar.activation(lnt[:], u_half[64:65, :], LN)
                rec = mp.tile([1, 4 * HALF], F32, tag="rec", name="rec")
                nc.scalar.activation(rec[:], lnt[:], EXP, scale=-1.0)
                for g in range(2):
                    for i in range(2):
                        s_slot = 2 * g + i
                        bc = mp.tile([64, HALF], F32, tag=f"bc{s_slot}",
                                     name="bc")
                        nc.gpsimd.partition_broadcast(
                            bc[:],
                            rec[0:1, s_slot * HALF:(s_slot + 1) * HALF])
                        nc.vector.tensor_mul(
                            zc[i][g * 64:(g + 1) * 64, :],
                            u_half[0:64,
                                   s_slot * HALF:(s_slot + 1) * HALF],
                            bc[:],
                        )

            def o_proj_st(zc, s0):
                # one 128-row block of the output projection + store
                o_sb = op.tile([P, D], F32, tag="ost", name="osb")
                st = (s0 // P) % 2
                for dsp in range(2):
                    o_ps = ps_sc.tile([P, SPAN], F32, tag="sc", name="ops")
                    for ch in range(2):
                        nc.tensor.matmul(
                            o_ps[:],
                            zc[ch][:, st * P:(st + 1) * P],
                            wo_sb[ch][:, dsp * SPAN:(dsp + 1) * SPAN],
                            start=(ch == 0),
                            stop=(ch == 1),
                        )
                    nc.vector.tensor_copy(
                        o_sb[:, dsp * SPAN:(dsp + 1) * SPAN], o_ps[:])
                nc.sync.dma_start(out_d[s0:s0 + P, :], o_sb[:])

            proj_left = [0] * (NSPAN + 1)

            def proj_thunks(sp):
                def wrap(fn):
                    def run():
                        proj_left[sp] -= 1
                        fn()
                    return run
                th = [wrap(lambda e=e: q_proj(sp, e)) for e in range(2)]
                th.append(wrap(lambda: k_proj(sp)))
                th += [wrap(lambda kt=kt: v_proj(kt))
                       for kt in range(4 * sp, 4 * sp + 4)]
                proj_left[sp] += len(th)
                return th

            fillq = deque()

            # span 0: only Q/K/V0 upfront so its first scores start ~5us
            # earlier; V1-3 ride the queue uncounted (pop rate guarantees
            # V(j) is emitted before iteration j+1 needs vaug[j])
            q_proj(0, 0)
            q_proj(0, 1)
            k_proj(0)
            v_proj(0)
            fillq.extend(lambda kt=kt: v_proj(kt) for kt in (1, 2, 3))

            for sp in range(NSPAN):
                # this span's own projections MUST be emitted before its
                # first score matmuls read qT/kT/vaug (emission order is
                # dataflow order for a fixed SBUF slice)
                while proj_left[sp] > 0:
                    fillq.popleft()()
                if sp + 1 < NSPAN:
                    fillq.extend(proj_thunks(sp + 1))

                q0 = sp * SPAN
                nkt = (q0 + SPAN) // P   # k tiles touching this span
                hb = nkt - 2             # k tiles touching the low half
                u_halves = [
                    ps_u.tile([P, 4 * HALF], F32, tag="u", name=f"u{h}")
                    for h in range(2)
                ]
                zcs = [
                    [zp.tile([P, HALF], BF, tag=f"zt{h}{c}", name=f"z{c}")
                     for c in range(2)]
                    for h in range(2)
                ]

                # AV of k-tile kt is emitted after the scores+exp of kt+1,
                # hiding the ACT exp latency from the PE stream; each AV
                # batch entry splits over the two half-span u tiles
                def emit_av(batch):
                    for g, i, e_sb, kt_, off_, w_ in batch:
                        s_slot = 2 * g + i
                        for h in range(2):
                            lo = max(off_ - h * HALF, 0)
                            hi = min(off_ + w_ - h * HALF, HALF)
                            if lo >= hi:
                                continue
                            last_kt = hb - 1 if h == 0 else nkt - 1
                            # two head slots share each PSUM bank and
                            # start=True clears has_written for the WHOLE
                            # bank: only the bank's first slot may set it
                            # (the sibling's first write lands on cleared
                            # bits, which means overwrite -> still correct)
                            nc.tensor.matmul(
                                u_halves[h][0:65,
                                            s_slot * HALF + lo:
                                            s_slot * HALF + hi],
                                vaug[kt_][:, g * 65:(g + 1) * 65],
                                e_sb[:, i * SPAN + h * HALF + lo:
                                     i * SPAN + h * HALF + hi],
                                start=(kt_ == 0 and i == 0),
                                stop=(kt_ == last_kt),
                                skip_group_check=True,
                            )

                pending = []
                for kt in range(nkt):
                    k0 = kt * P
                    off = max(k0 - q0, 0)
                    w = SPAN - off
                    cur = []
                    for g in range(2):
                        # both i-heads of group g share one 2-bank PSUM
                        # tile -> a single exp instruction covers them
                        pair = ps_sc.tile([P, 2 * SPAN], F32, tag="sc",
                                          name=f"pair{g}")
                        for i in range(2):
                            nc.tensor.matmul(
                                pair[:, i * SPAN + off:i * SPAN + off + w],
                                kT[g * 64:(g + 1) * 64, k0:k0 + P],
                                qT[i][g * 64:(g + 1) * 64,
                                      q0 + off:q0 + off + w],
                                start=True,
                                stop=True,
                            )
                        e_sb = ep.tile([P, 2 * SPAN], BF, tag="e",
                                       name=f"e{g}")
                        pv = pair.rearrange("p (i w) -> p i w", i=2)
                        ev = e_sb.rearrange("p (i w) -> p i w", i=2)
                        nc.scalar.activation(
                            ev[:, :, off:off + w], pv[:, :, off:off + w],
                            EXP, scale=0.125,
                        )
                        if k0 >= q0:  # diagonal tile -> causal mask
                            mv = mask.unsqueeze(1).broadcast_to([P, 2, P])
                            nc.vector.tensor_mul(
                                ev[:, :, off:off + P],
                                ev[:, :, off:off + P],
                                mv,
                            )
                        cur.append((g, 0, e_sb, kt, off, w))
                        cur.append((g, 1, e_sb, kt, off, w))
                    emit_av(pending)
                    pending = cur
                    if kt == hb:
                        # low half's AV chain (k tiles 0..hb-1) just went
                        # out -> emit its normalize now, queue its O-proj
                        normalize(u_halves[0], zcs[0])
                        fillq.append(
                            lambda z=zcs[0], s=q0: o_proj_st(z, s))
                        fillq.append(
                            lambda z=zcs[0], s=q0 + P: o_proj_st(z, s))
                    if fillq:
                        fillq.popleft()()
                emit_av(pending)

                hq1 = q0 + HALF
                if sp + 1 < NSPAN:
                    fillq.append(
                        lambda u=u_halves[1], z=zcs[1]: normalize(u, z))
                    fillq.append(lambda z=zcs[1], s=hq1: o_proj_st(z, s))
                    fillq.append(
                        lambda z=zcs[1], s=hq1 + P: o_proj_st(z, s))
                else:
                    normalize(u_halves[1], zcs[1])
                    while fillq:
                        fillq.popleft()()
                    o_proj_st(zcs[1], hq1)
                    o_proj_st(zcs[1], hq1 + P)

    nc.finalize()
    return nc


def kernel(resid, W_Q, W_K, W_V, W_out, b_out):
    global LAST_RESULTS, _CACHED_NC
    resid = np.asarray(resid, np.float32)
    W_Q = np.asarray(W_Q, np.float32)
    W_K = np.asarray(W_K, np.float32)
    W_V = np.asarray(W_V, np.float32)
    W_out = np.asarray(W_out, np.float32)
    b_out = np.asarray(b_out, np.float32)

    if _CACHED_NC is None:
        _CACHED_NC = _build_program()
    nc = _CACHED_NC

    bf = ml_dtypes.bfloat16
    residT = [np.ascontiguousarray(resid[b].T).astype(bf) for b in range(2)]
    in_maps = []
    for c in range(8):
        b, q = c // 4, c % 4
        # interleaved head order [h0, h2, h1, h3]: storage slot (g, i) holds
        # local head 2g+i -> qT[i]/zc[i] rows g*64 (see _build_program)
        heads = [4 * q, 4 * q + 2, 4 * q + 1, 4 * q + 3]
        groups = [2 * q, 2 * q + 1]
        in_maps.append({
            "resid_t": residT[b],
            "wq": np.ascontiguousarray(
                W_Q[:, heads, :].reshape(D, 256)).astype(bf),
            "wk": np.ascontiguousarray(
                W_K[:, groups, :].reshape(D, 128)).astype(bf),
            "wv": np.ascontiguousarray(
                W_V[:, groups, :].reshape(D, 128)).astype(bf),
            "wo": np.ascontiguousarray(
                W_out[:, heads, :].transpose(1, 0, 2).reshape(256, D)
            ).astype(bf),
        })

    res = run_bass_kernel_spmd(nc, in_maps, core_ids=list(range(8)))
    LAST_RESULTS = res

    out = np.zeros((2, S, D), np.float32)
    for c in range(8):
        out[c // 4] += res.results[c]["out"]
    out += b_out
    return out
